# revision 18
# baseline (speedup 1.0000x reference)
"""Trainium2 Bass kernel for nn_Encoder_54915451847178 (6-layer dense
transformer encoder, no-softmax attention, 2D layernorm).

Strategy: data-parallel over batch (256 -> 32 samples per NeuronCore x 8).
On each core, activations live in SBUF feature-major (hT: [D partitions, T
tokens]) as float32r (TF32-like, full PE rate at N>=256, ~1e-4 matmul err).
Residual stream is updated in place; layernorm stats via DVE free-axis
reduces + a ones-vector matmul for the partition reduction.

Self-contained: hardcodes all shapes; only needs numpy/jax/concourse
(environment-provided) at run time.
"""
import math

import numpy as np

import concourse.bass as bass
import concourse.tile as tile
from concourse import mybir, bacc
from concourse.bass import IndirectOffsetOnAxis
from concourse.bass_utils import run_bass_kernel_spmd
from concourse.masks import make_identity

F32 = mybir.dt.float32
F32R = mybir.dt.float32r
BF16 = mybir.dt.bfloat16
I32 = mybir.dt.int32
AF = mybir.ActivationFunctionType
ALU = mybir.AluOpType
AX = mybir.AxisListType

# model dims (hardcoded from the problem spec)
D_MODEL = 512
N_LAYERS = 6
N_HEADS = 8
INNER = 2048
VOCAB = 2048
SEQ = 128
BATCH = 256
HEAD_DIM = 64
EPS = 1e-5
N_CORES = 8
BC = BATCH // N_CORES          # samples per core = 32
T = BC * SEQ                   # tokens per core = 4096
DT = D_MODEL // 128            # 4 feature tiles
IT = INNER // 128              # 16 inner tiles
NCH = BC // 4                  # 8 chunks of 4 samples (512 tokens)
CHS = 4 * SEQ                  # chunk token count = 512
LN_N = float(SEQ * D_MODEL)    # layernorm normalization count

# dtype config: all matmul operands in f32r (TF32-like precision, full rate)
MM_DT = F32R
AT_DT = F32R
FF_DT = F32R


def _bcast3(ap, reps):
    """[P, n] AP -> [P, n, reps] view repeating each element along a new axis."""
    a = ap
    return bass.AP(tensor=a.tensor, offset=a.offset,
                   ap=[list(a.ap[0]), list(a.ap[1]), [0, reps]])


def _build_nc():
    nc = bacc.Bacc("TRN2", target_bir_lowering=False, debug=False)

    # ---- DRAM I/O ----
    x_idx = nc.dram_tensor("x_idx", [BC, SEQ], I32, kind="ExternalInput").ap()
    emb = nc.dram_tensor("emb", [VOCAB, D_MODEL], F32, kind="ExternalInput").ap()
    pet = nc.dram_tensor("pet", [128, DT, SEQ], F32, kind="ExternalInput").ap()
    wq = nc.dram_tensor("wq", [N_LAYERS, DT, DT, 128, 128], MM_DT, kind="ExternalInput").ap()
    wk = nc.dram_tensor("wk", [N_LAYERS, DT, DT, 128, 128], MM_DT, kind="ExternalInput").ap()
    wv = nc.dram_tensor("wv", [N_LAYERS, DT, 128, D_MODEL], MM_DT, kind="ExternalInput").ap()
    w1 = nc.dram_tensor("w1", [N_LAYERS, DT, IT, 128, 128], MM_DT, kind="ExternalInput").ap()
    w2 = nc.dram_tensor("w2", [N_LAYERS, IT, DT, 128, 128], MM_DT, kind="ExternalInput").ap()
    bq = nc.dram_tensor("bq", [N_LAYERS, DT, 128], F32, kind="ExternalInput").ap()
    bk = nc.dram_tensor("bk", [N_LAYERS, DT, 128], F32, kind="ExternalInput").ap()
    bv = nc.dram_tensor("bv", [N_LAYERS, D_MODEL], F32R, kind="ExternalInput").ap()
    b1 = nc.dram_tensor("b1", [N_LAYERS, IT, 128], F32, kind="ExternalInput").ap()
    b2 = nc.dram_tensor("b2", [N_LAYERS, DT, 128], F32, kind="ExternalInput").ap()
    out = nc.dram_tensor("out", [BC, SEQ, D_MODEL], F32, kind="ExternalOutput").ap()

    with tile.TileContext(nc) as tc:
        with (
            tc.tile_pool(name="persist", bufs=1) as persist,
            tc.tile_pool(name="wpool", bufs=1) as wpool,
            tc.tile_pool(name="qk", bufs=2) as qkpool,
            tc.tile_pool(name="vv", bufs=1) as vpool,
            tc.tile_pool(name="sc", bufs=1) as scpool,
            tc.tile_pool(name="z1", bufs=2) as z1pool,
            tc.tile_pool(name="t2", bufs=1) as t2pool,
            tc.tile_pool(name="xb", bufs=1) as xbpool,
            tc.tile_pool(name="pt", bufs=2) as ptpool,
            tc.tile_pool(name="st", bufs=2) as stpool,
            tc.tile_pool(name="ot", bufs=1) as otpool,
            tc.tile_pool(name="pmm", bufs=2, space="PSUM") as pmm,
            tc.tile_pool(name="psm", bufs=2, space="PSUM") as psm,
        ):
            # ---- persistent SBUF ----
            h = [[persist.tile([128, CHS], F32R, tag=f"h{d}_{c}", name=f"h{d}_{c}")
                  for c in range(NCH)] for d in range(DT)]
            ident32 = persist.tile([128, 128], F32, tag="ident32")
            make_identity(nc, ident32[:])
            identr = persist.tile([128, 128], F32R, tag="identr")
            nc.vector.tensor_copy(identr[:], ident32[:])
            ones32 = persist.tile([128, 128], F32, tag="ones32")
            nc.vector.memset(ones32[:], 1.0)
            ones_mat = persist.tile([128, 128], F32R, tag="onesm")
            nc.vector.tensor_copy(ones_mat[:], ones32[:])
            ones_row = persist.tile([1, 128], F32R, tag="onesr")
            nc.vector.tensor_copy(ones_row[:], ones32[0:1, :])
            eps_t = persist.tile([128, 1], F32, tag="eps")
            nc.vector.memset(eps_t[:], EPS)

            xs = persist.tile([128, BC], I32, tag="xs")
            nc.sync.dma_start(xs[:], x_idx.rearrange("b s -> s b"))
            pet_s = persist.tile([128, DT, SEQ], F32, tag="pet")
            nc.sync.dma_start(pet_s[:], pet[:])
            bq_s = persist.tile([128, N_LAYERS, DT], F32, tag="bq")
            nc.sync.dma_start(bq_s[:], bq.rearrange("l m p -> p l m"))
            bk_s = persist.tile([128, N_LAYERS, DT], F32, tag="bk")
            nc.sync.dma_start(bk_s[:], bk.rearrange("l m p -> p l m"))
            bv_s = persist.tile([1, N_LAYERS, D_MODEL], F32R, tag="bv")
            nc.sync.dma_start(bv_s[:], bv[:].unsqueeze(0))
            b1_s = persist.tile([128, N_LAYERS, IT], F32, tag="b1")
            nc.sync.dma_start(b1_s[:], b1.rearrange("l m p -> p l m"))
            b2_s = persist.tile([128, N_LAYERS, DT], F32, tag="b2")
            nc.sync.dma_start(b2_s[:], b2.rearrange("l m p -> p l m"))

            # ---- embedding gather + transpose to feature-major (+pos enc) ----
            for b in range(BC):
                g = otpool.tile([128, D_MODEL], F32, tag="ot", name="g")
                nc.gpsimd.indirect_dma_start(
                    out=g[:], out_offset=None, in_=emb[:],
                    in_offset=IndirectOffsetOnAxis(ap=xs[:, b:b + 1], axis=0),
                )
                ch, bo = b // 4, (b % 4) * SEQ
                for d in range(DT):
                    tp = psm.tile([128, 128], F32, tag="scp", bufs=2, name="tpp")
                    nc.tensor.transpose(tp[:], g[:, d * 128:(d + 1) * 128], ident32[:])
                    nc.vector.tensor_tensor(
                        out=h[d][ch][:, bo:bo + SEQ], in0=tp[:],
                        in1=pet_s[:, d, :], op=ALU.add)

            # ---- layernorm over (S, D): stats+apply from fp32 xb, writes h ----
            def layernorm(ch, xb):
                pt = ptpool.tile([128, 2, 4, DT], F32R, tag="pt")
                for d in range(DT):
                    hv = xb[:, d, :].rearrange("p (s c) -> p s c", c=SEQ)
                    sq = t2pool.tile([128, CHS], F32, tag="t2", name="sq")
                    nc.vector.tensor_tensor(out=sq[:], in0=hv, in1=hv, op=ALU.mult)
                    with nc.allow_low_precision(reason="f32r partials feed stats matmul"):
                        nc.vector.tensor_reduce(
                            out=pt[:, 0, :, d], in_=hv, axis=AX.X, op=ALU.add)
                        nc.vector.tensor_reduce(
                            out=pt[:, 1, :, d],
                            in_=sq[:].rearrange("p (s c) -> p s c", c=SEQ),
                            axis=AX.X, op=ALU.add)
                stp = psm.tile([128, 32], F32, tag="scp", bufs=2, name="stp")
                nc.tensor.matmul(stp[:], ones_mat[:], pt[:].rearrange("p a b c -> p (a b c)"),
                                 start=True, stop=True)
                sums = stpool.tile([128, 32], F32, tag="sums")
                nc.vector.tensor_copy(sums[:], stp[:])
                tot = stpool.tile([128, 8], F32, tag="tot")
                nc.vector.tensor_reduce(
                    out=tot[:].rearrange("p (a b) -> p a b", a=2),
                    in_=sums[:].rearrange("p (a s d) -> p (a s) d", a=2, s=4),
                    axis=AX.X, op=ALU.add)
                negm = stpool.tile([128, 4], F32, tag="negm")
                nc.scalar.mul(negm[:], tot[:, 0:4], -1.0 / LN_N)
                e2 = stpool.tile([128, 4], F32, tag="e2")
                nc.scalar.mul(e2[:], tot[:, 4:8], 1.0 / LN_N)
                var = stpool.tile([128, 4], F32, tag="var")
                nc.vector.tensor_tensor(out=var[:], in0=negm[:], in1=negm[:], op=ALU.mult)
                nc.vector.tensor_tensor(out=var[:], in0=e2[:], in1=var[:], op=ALU.subtract)
                std = stpool.tile([128, 4], F32, tag="std")
                nc.scalar.activation(std[:], var[:], AF.Sqrt, bias=eps_t[:])
                rstd = stpool.tile([128, 4], F32, tag="rstd")
                nc.vector.reciprocal(rstd[:], std[:])
                negm_b = _bcast3(negm[:], SEQ)
                rstd_b = _bcast3(rstd[:], SEQ)
                for d in range(DT):
                    xv = xb[:, d, :].rearrange("p (s c) -> p s c", c=SEQ)
                    hw = h[d][ch][:].rearrange("p (s c) -> p s c", c=SEQ)
                    nc.vector.tensor_tensor(out=xv, in0=xv, in1=negm_b, op=ALU.add)
                    nc.vector.tensor_tensor(out=hw, in0=xv, in1=rstd_b, op=ALU.mult)

            # ---- transformer layers ----
            for l in range(N_LAYERS):
                wq_t = [[wpool.tile([128, 128], MM_DT, tag=f"wq{k}_{m}", name=f"wq{k}_{m}")
                         for m in range(DT)] for k in range(DT)]
                wk_t = [[wpool.tile([128, 128], MM_DT, tag=f"wk{k}_{m}", name=f"wk{k}_{m}")
                         for m in range(DT)] for k in range(DT)]
                wv_t = [wpool.tile([128, D_MODEL], MM_DT, tag=f"wv{k}", name=f"wv{k}")
                        for k in range(DT)]
                w1_t = [[wpool.tile([128, 128], MM_DT, tag=f"w1{k}_{m}", name=f"w1{k}_{m}")
                         for m in range(IT)] for k in range(DT)]
                w2_t = [[wpool.tile([128, 128], MM_DT, tag=f"w2{k}_{m}", name=f"w2{k}_{m}")
                         for m in range(DT)] for k in range(IT)]
                for k in range(DT):
                    for m in range(DT):
                        nc.sync.dma_start(wq_t[k][m][:], wq[l, k, m])
                        nc.sync.dma_start(wk_t[k][m][:], wk[l, k, m])
                    nc.sync.dma_start(wv_t[k][:], wv[l, k])
                    for m in range(IT):
                        nc.sync.dma_start(w1_t[k][m][:], w1[l, k, m])
                for k in range(IT):
                    for m in range(DT):
                        nc.sync.dma_start(w2_t[k][m][:], w2[l, k, m])

                for ch in range(NCH):
                    # V projection (token-major) with fused bias
                    vt = vpool.tile([128, 4, D_MODEL], AT_DT, tag="v")
                    for b4 in range(4):
                        ps = pmm.tile([128, D_MODEL], F32, tag="pmm")
                        for k in range(DT):
                            nc.tensor.matmul(
                                ps[:], h[k][ch][:, b4 * SEQ:(b4 + 1) * SEQ], wv_t[k][:],
                                start=(k == 0), stop=False)
                        nc.tensor.matmul(ps[:], ones_row[:], bv_s[:1, l, :],
                                         start=False, stop=True)
                        nc.any.tensor_copy(vt[:, b4, :], ps[:])
                    # Q/K per head-pair + attention, residual-added into xbuf
                    xb = xbpool.tile([128, DT, CHS], F32, tag="xb")
                    for m in range(DT):
                        qt = qkpool.tile([128, CHS], AT_DT, tag="q")
                        kt_ = qkpool.tile([128, CHS], AT_DT, tag="k")
                        for dst, wt, bt in ((qt, wq_t, bq_s), (kt_, wk_t, bk_s)):
                            ps = pmm.tile([128, CHS], F32, tag="pmm")
                            for k in range(DT):
                                nc.tensor.matmul(ps[:], wt[k][m][:], h[k][ch][:],
                                                 start=(k == 0), stop=(k == DT - 1))
                            nc.scalar.activation(dst[:], ps[:], AF.Identity,
                                                 bias=bt[:, l, m:m + 1])
                        for b4 in range(4):
                            bo = b4 * SEQ
                            for hh in (2 * m, 2 * m + 1):
                                po = (hh % 2) * 64
                                scp = psm.tile([128, 128], F32, tag="scp")
                                nc.tensor.matmul(
                                    scp[:], kt_[po:po + 64, bo:bo + SEQ],
                                    qt[po:po + 64, bo:bo + SEQ], start=True, stop=True)
                                scs = scpool.tile([128, 128], AT_DT, tag="scs")
                                nc.any.tensor_copy(scs[:], scp[:])
                                atp = psm.tile([64, 128], F32, tag="scp", bufs=2, name="atp")
                                nc.tensor.matmul(
                                    atp[:], vt[:, b4, hh * 64:(hh + 1) * 64], scs[:],
                                    start=True, stop=True)
                                hsl = h[m][ch][po:po + 64, bo:bo + SEQ]
                                nc.vector.tensor_tensor(
                                    out=xb[po:po + 64, m, bo:bo + SEQ],
                                    in0=atp[:], in1=hsl.bitcast(F32), op=ALU.add)
                    layernorm(ch, xb)
                    # FFN (f32r): z1 per-ki tile; z2 accumulates in 4 psum banks
                    z2ps = [pmm.tile([128, CHS], F32, tag=f"z2p{m}", bufs=1,
                                     name=f"z2p{m}") for m in range(DT)]
                    for ki in range(IT):
                        ps = pmm.tile([128, CHS], F32, tag="pmm")
                        for k in range(DT):
                            nc.tensor.matmul(ps[:], w1_t[k][ki][:], h[k][ch][:],
                                             start=(k == 0), stop=(k == DT - 1))
                        z1 = z1pool.tile([128, CHS], FF_DT, tag="z1")
                        nc.scalar.activation(z1[:], ps[:], AF.Relu,
                                             bias=b1_s[:, l, ki:ki + 1])
                        for m in range(DT):
                            nc.tensor.matmul(z2ps[m][:], w2_t[ki][m][:], z1[:],
                                             start=(ki == 0), stop=(ki == IT - 1))
                    xb2 = xbpool.tile([128, DT, CHS], F32, tag="xb")
                    for m in range(DT):
                        t2 = t2pool.tile([128, CHS], F32, tag="t2")
                        nc.scalar.activation(t2[:], z2ps[m][:], AF.Identity,
                                             bias=b2_s[:, l, m:m + 1])
                        nc.vector.tensor_tensor(
                            out=xb2[:, m, :], in0=t2[:],
                            in1=h[m][ch][:].bitcast(F32), op=ALU.add)
                    layernorm(ch, xb2)

            # ---- output: transpose back to token-major and store ----
            for b in range(BC):
                ch, bo = b // 4, (b % 4) * SEQ
                ot = otpool.tile([128, D_MODEL], F32, tag="ot")
                for d in range(DT):
                    tp = psm.tile([128, 128], F32R, tag="scp", bufs=2, name="tpr")
                    nc.tensor.matmul(tp[:], h[d][ch][:, bo:bo + SEQ], identr[:],
                                     is_transpose=True, start=True, stop=True)
                    nc.vector.tensor_copy(ot[:, d * 128:(d + 1) * 128], tp[:])
                nc.sync.dma_start(out[b], ot[:])

    nc.compile()
    return nc


_NC_CACHE = {}


def _get_nc():
    if "nc" not in _NC_CACHE:
        _NC_CACHE["nc"] = _build_nc()
    return _NC_CACHE["nc"]


def _pos_encoding():
    pos = np.arange(SEQ, dtype=np.float64)[:, None]
    i = np.arange(D_MODEL // 2, dtype=np.float64)[None, :]
    theta = pos / np.power(10000.0, 2.0 * i / D_MODEL)
    pe = np.stack([np.sin(theta), np.cos(theta)], axis=-1).reshape(SEQ, D_MODEL)
    return pe.astype(np.float32)


def _prep_inputs(x, emb, Wq, bq, Wk, bk, Wv, bv, W1, b1, W2, b2):
    scale = HEAD_DIM ** -0.5
    x = np.asarray(x).astype(np.int32).reshape(N_CORES, BC, SEQ)
    emb = np.ascontiguousarray(np.asarray(emb, np.float32))
    pe = _pos_encoding()                                   # [S, D]
    pet = np.ascontiguousarray(
        pe.T.reshape(DT, 128, SEQ).transpose(1, 0, 2))     # [128, DT, S]

    def tile_dm(w):  # [L, A, B] -> [L, A/128, B/128, 128, 128] (k-tiles, m-tiles)
        L, A, B = w.shape
        return np.ascontiguousarray(
            w.reshape(L, A // 128, 128, B // 128, 128).transpose(0, 1, 3, 2, 4))

    Wq = np.asarray(Wq, np.float32)   # [L, H, D, E]
    Wk = np.asarray(Wk, np.float32)
    Wv = np.asarray(Wv, np.float32)
    wq_f = Wq.transpose(0, 2, 1, 3).reshape(N_LAYERS, D_MODEL, D_MODEL) * scale
    wk_f = Wk.transpose(0, 2, 1, 3).reshape(N_LAYERS, D_MODEL, D_MODEL)
    wv_f = Wv.transpose(0, 2, 1, 3).reshape(N_LAYERS, D_MODEL, D_MODEL)
    wq_t = tile_dm(wq_f)
    wk_t = tile_dm(wk_f)
    wv_t = np.ascontiguousarray(
        wv_f.reshape(N_LAYERS, DT, 128, D_MODEL))          # [L, DT, 128, D]
    w1_t = tile_dm(np.asarray(W1, np.float32))
    w2_t = tile_dm(np.asarray(W2, np.float32))

    bq_f = (np.asarray(bq, np.float32).reshape(N_LAYERS, D_MODEL) * scale
            ).reshape(N_LAYERS, DT, 128)
    bk_f = np.asarray(bk, np.float32).reshape(N_LAYERS, DT, 128)
    bv_f = np.asarray(bv, np.float32).reshape(N_LAYERS, D_MODEL)
    b1_f = np.asarray(b1, np.float32).reshape(N_LAYERS, IT, 128)
    b2_f = np.asarray(b2, np.float32).reshape(N_LAYERS, DT, 128)

    common = dict(emb=emb, pet=pet, wq=wq_t, wk=wk_t, wv=wv_t, w1=w1_t, w2=w2_t,
                  bq=bq_f, bk=bk_f, bv=bv_f, b1=b1_f, b2=b2_f)
    return [dict(common, x_idx=np.ascontiguousarray(x[c])) for c in range(N_CORES)]


def kernel(**inputs):
    nc = _get_nc()
    in_maps = _prep_inputs(**inputs)
    r = run_bass_kernel_spmd(nc, in_maps, core_ids=list(range(N_CORES)))
    return np.concatenate([r.results[c]["out"] for c in range(N_CORES)], axis=0)
